# revision 25
# baseline (speedup 1.0000x reference)
"""AFT-local attention on 8 Trainium2 NeuronCores.

Reference (per batch element b, S=2048, D=512, window=128):
    query = q @ Wq.T + bq;  Q_ = sigmoid(query)
    key_p = k @ Wk.T + bk;  ek = exp(key_p)
    value = v @ Wv.T + bv;  ekv = ek * value
    ew    = exp(w_bias * local_mask)          # S x S, == 1 outside the band
    num_raw = ew @ ekv ; den = ew @ ek        # dense S x S einsums
    num  = Q_ * num_raw;  x = num / den
    out1 = x @ out_w.T + out_b
    return (out1, num)

Key decomposition: ew = 1 + (exp(wb_masked) - 1) restricted to the band
|i-j| < 128, so  ew @ Y = colsum(Y) + EWM1_band @ Y  where the banded part
only touches <=3 column tiles of 128 per row tile of 128 (46 block matmuls
instead of 256 dense ones).

Sharding: pure data-parallel; batch B=8 -> one batch element per core.

Implementation notes (v2, all rates measured on this silicon):
- PE streams 1 column/cycle at 2.4GHz for EVERY dtype (f16 == bf16 == f32r
  at N=512: ~216ns/MM measured).  PE work is therefore minimized by column
  count, not dtype: ~352 N=512 matmuls + 64 transposes ~= 78us/core floor.
- ewm1 = exp(band)-1 is computed ON HOST and shipped as f16 (values are
  tiny biases |x|<0.04): kills the on-chip ACT exp + DVE add prep of v1.
- ekk ([ekv|ek]) is f16: halves its SBUF/HBM footprint; band matmuls are
  f16; colsum error from f16 rounding averages out over 2048 terms.
- DMA instructions cost ~600-850ns of queue occupancy EACH (size-almost-
  independent), so inputs are shipped as ONE descriptor per [128, >=1kB]
  block: k|v merged per tile, whole weight matrices, the whole band.
  3 queues: sync (most inputs + num out), scalar (2 early loads only),
  gpsimd (out1 + no inputs).
- The S-direction colsum accumulation chain is split DVE/gpsimd by tile
  parity so phase B stays PE-paced.
- Phase C epilogue: den/num PSUM evictions fused with the colsum add on
  DVE, reciprocal_approx_fast on DVE, x = num*rcp on GPSIMD writing bf16;
  transposes + output projection run bf16 (1.0 vs 1.5 cyc/row transpose).
- PSUM evictions (xT, out) on ACT; software-pipelined tail as in v1.
"""

import sys

if "/opt/trn_rl_repo" not in sys.path:
    sys.path.insert(0, "/opt/trn_rl_repo")

import numpy as np

import concourse.bacc as bacc
import concourse.mybir as mybir
import concourse.tile as tile
from concourse.bass import ts

from concourse.bass_utils import run_bass_kernel_spmd
from concourse.masks import make_identity

F32 = mybir.dt.float32
F32R = mybir.dt.float32r
BF16 = mybir.dt.bfloat16
F16 = mybir.dt.float16
EXP = mybir.ActivationFunctionType.Exp
SIGMOID = mybir.ActivationFunctionType.Sigmoid

S = 2048
D = 512
P = 128
NT = S // P  # 16 sequence tiles
NC = D // P  # 4 contraction chunks of the model dim
N_CORES = 8


def _band_blocks(i):
    """Valid (jl, j) column-tile neighbors for row tile i."""
    return [(jl, i - 1 + jl) for jl in range(3) if 0 <= i - 1 + jl < NT]


def build(with_biases):
    nc = bacc.Bacc(None, target_bir_lowering=False, debug=False)

    # host-packed layouts: every [128, ...] DMA partition line is contiguous
    kv_d = nc.dram_tensor("kvP", [NT, P, 2 * NC * P], F16, kind="ExternalInput")
    qP_d = nc.dram_tensor("qP", [NT, P, NC * P], F16, kind="ExternalInput")
    wq_d = nc.dram_tensor("wqP", [P, NC * D], F16, kind="ExternalInput")
    wk_d = nc.dram_tensor("wkP", [P, NC * D], F16, kind="ExternalInput")
    wv_d = nc.dram_tensor("wvP", [P, NC * D], F16, kind="ExternalInput")
    wo_d = nc.dram_tensor("woP", [P, NC * D], BF16, kind="ExternalInput")
    band_d = nc.dram_tensor("bandP", [P, NT * 3 * P], F16, kind="ExternalInput")
    if with_biases:
        # rows: bq, bk, bv, bo
        bias_d = nc.dram_tensor("biases", [4, D], F32, kind="ExternalInput")
    out1_d = nc.dram_tensor("out1", [S, D], F32, kind="ExternalOutput")
    num_d = nc.dram_tensor("num", [S, D], F32, kind="ExternalOutput")

    with tile.TileContext(nc) as tc:
        with (
            tc.tile_pool(name="consts", bufs=1) as consts,
            tc.tile_pool(name="weights", bufs=1) as wpool,
            tc.tile_pool(name="ekk_pool", bufs=1) as ekkpool,
            tc.tile_pool(name="kv_in", bufs=1) as kvpool,
            tc.tile_pool(name="q_in", bufs=1) as qpool,
            tc.tile_pool(name="work", bufs=3) as work,
            tc.tile_pool(name="outs", bufs=3) as outs,
            tc.tile_pool(name="psum", bufs=1, space="PSUM") as psum,
        ):
            # ---- input staging.  Emission order = per-queue issue order, so
            # the first matmul's operands (kv tile 0, wk chunk 0) lead their
            # queues: wk on sync, kv0 on scalar ----
            w_sb = {}
            kv_tiles = []

            def emit_kv(j, eng, split=False):
                t = kvpool.tile([P, 2, NC, P], F16, tag="kv", bufs=6, name="kv_t")
                src = kv_d[j].rearrange("p (u c t) -> p u c t", u=2, c=NC)
                if split:  # k-half lands first so the lead matmul starts early
                    eng.dma_start(t[:, 0], src[:, 0])
                    eng.dma_start(t[:, 1], src[:, 1])
                else:
                    eng.dma_start(t, src)
                kv_tiles.append(t)

            # critical-path order on the single input queue: kv0 k-half +
            # wk chunk 0 feed the first matmul; v0/wv follow ~1.7us later
            wk_t = wpool.tile([P, NC, D], F16, tag="w_wk", name="w_wk")
            wk_src = wk_d[:, :].rearrange("p (c n) -> p c n", c=NC)
            kv0 = kvpool.tile([P, 2, NC, P], F16, tag="kv", bufs=6, name="kv_t")
            kv0_src = kv_d[0].rearrange("p (u c t) -> p u c t", u=2, c=NC)
            nc.sync.dma_start(kv0[:, 0], kv0_src[:, 0])
            nc.sync.dma_start(wk_t[:, 0, :], wk_src[:, 0, :])
            wv_t = wpool.tile([P, NC, D], F16, tag="w_wv", name="w_wv")
            wv_src = wv_d[:, :].rearrange("p (c n) -> p c n", c=NC)
            nc.sync.dma_start(kv0[:, 1], kv0_src[:, 1])
            nc.sync.dma_start(wv_t[:, 0, :], wv_src[:, 0, :])
            nc.sync.dma_start(wk_t[:, 1:, :], wk_src[:, 1:, :])
            nc.sync.dma_start(wv_t[:, 1:, :], wv_src[:, 1:, :])
            w_sb["wk"] = wk_t
            w_sb["wv"] = wv_t
            kv_tiles.append(kv0)

            emit_kv(1, nc.sync, split=True)
            emit_kv(2, nc.sync)

            # whole banded exp(w_bias)-1, host-precomputed, one descriptor
            ewm1 = wpool.tile([P, NT, 3 * P], F16, tag="ewm1", name="ewm1")
            nc.sync.dma_start(
                ewm1, band_d[:, :].rearrange("p (i f) -> p i f", i=NT)
            )

            wq_t = wpool.tile([P, NC, D], F16, tag="w_wq", name="w_wq")
            nc.sync.dma_start(
                wq_t, wq_d[:, :].rearrange("p (c n) -> p c n", c=NC)
            )
            w_sb["wq"] = wq_t
            wo_t = wpool.tile([P, NC, D], BF16, tag="w_wo", name="w_wo")
            nc.sync.dma_start(
                wo_t, wo_d[:, :].rearrange("p (c n) -> p c n", c=NC)
            )
            w_sb["wo"] = wo_t
            if with_biases:
                bias_sb = consts.tile([4, D], F32R)
                nc.sync.dma_start(bias_sb, bias_d[:, :].bitcast(F32R))

            # ---- constants (gpsimd + vector lead with these; neither queue
            # has early DMA duties in this schedule) ----
            identity_f32 = consts.tile([P, P], F32)
            make_identity(nc, identity_f32)
            identity_bf = consts.tile([P, P], BF16)
            nc.vector.tensor_copy(identity_bf, identity_f32)
            ones_f32 = consts.tile([P, 1], F32)
            nc.gpsimd.memset(ones_f32, 1.0)
            ones_col = consts.tile([P, 1], F32R)  # lhsT for column sums
            nc.vector.tensor_copy(ones_col, ones_f32)
            ones_row = consts.tile([1, P], F32R)  # lhsT for partition bcast
            nc.vector.tensor_copy(ones_row, ones_f32[0:1, 0:1].broadcast_to([1, P]))

            # ---- ekk: per seq-tile j, [ekv | ek] along free dim, f16 ----
            ekk = ekkpool.tile([P, NT, 2 * D], F16)

            # PE p-state warm-up: the engine needs ~3.4us of activity to
            # reach 2.4GHz; burn the input-DMA wait window on dummy matmuls
            # so the first real projections run at full clock
            warm_ps = psum.tile([P, P], F32, tag="bps", bufs=3, name="warm")
            for r in range(28):
                nc.tensor.matmul(
                    warm_ps[0:1, 0:64],
                    identity_bf[:, 0:1],
                    identity_bf[:, 0:64],
                    start=(r == 0),
                    stop=(r == 27),
                )

            def proj_psum(xT_tile, wname, bias_row):
                """psum [128, 512] = (x @ W.T + b) for one 128-seq tile."""
                ps = psum.tile([P, D], F32, tag="ps", bufs=3, name="proj_ps")
                for c in range(NC):
                    nc.tensor.matmul(
                        ps,
                        xT_tile[:, c, :],
                        w_sb[wname][:, c, :],
                        start=(c == 0),
                        stop=(c == NC - 1 and bias_row is None),
                    )
                if bias_row is not None:
                    nc.tensor.matmul(
                        ps,
                        ones_row[:, :],
                        bias_sb[bias_row : bias_row + 1, :],
                        start=False,
                        stop=True,
                    )
                return ps

            # ---- phase B: ek / ekv + column-sum chain (DVE/gpsimd split) ----
            # single accumulator, split by COLUMN range: gpsimd sums the
            # first 640 columns, DVE the last 384, every tile -- a flat
            # ~1.4us/tile on each engine instead of 2.3us spikes by parity
            acc = wpool.tile([P, 2 * D], F32R, tag="acc", name="acc")
            nc.gpsimd.memset(acc.bitcast(F32), 0.0)
            CSPLIT = 640
            q_tiles = []
            for j in range(NT):
                if j + 3 < NT:
                    emit_kv(j + 3, nc.sync)
                kvt = kv_tiles[j]

                keyp_ps = proj_psum(kvt[:, 0], "wk", 1 if with_biases else None)
                ek_view = ekk[:, j, D : 2 * D]
                nc.scalar.activation(ek_view, keyp_ps, EXP)

                val_ps = proj_psum(kvt[:, 1], "wv", 2 if with_biases else None)
                nc.vector.tensor_mul(ekk[:, j, 0:D], ek_view, val_ps)

                # column-sum accumulation, column-split gpsimd/DVE
                src = ekk[:, j, :]
                nc.gpsimd.tensor_add(
                    acc[:, 0:CSPLIT], acc[:, 0:CSPLIT], src[:, 0:CSPLIT]
                )
                nc.vector.tensor_add(
                    acc[:, CSPLIT:], acc[:, CSPLIT:], src[:, CSPLIT:]
                )

                if j >= NT - 3:  # q0..q2 prefetch at tail of phase B
                    i0 = j - (NT - 3)
                    qt = qpool.tile([P, NC, P], F16, tag="qT", bufs=4, name="qT_t")
                    nc.sync.dma_start(
                        qt, qP_d[i0].rearrange("p (c t) -> p c t", c=NC)
                    )
                    q_tiles.append(qt)

            # partition-reduce on the PE (ones^T @ acc), evict on ACT, then
            # broadcast back over 128 partitions via a K=1 matmul
            csum_bc = wpool.tile([P, 2 * D], F32, tag="csum_bc", name="csum_bc")

            def emit_csum_reduce():
                # both partial accs reduced straight into one PSUM tile --
                # no DVE combine on the B->C critical path
                for h in range(2):
                    cs_ps = psum.tile([P, D], F32, tag="xT", bufs=2, name=f"cs{h}")
                    nc.tensor.matmul(
                        cs_ps[0:1, :],
                        ones_col[:, :],
                        acc[:, h * D : (h + 1) * D],
                        start=True,
                        stop=True,
                    )
                    csr = consts.tile([1, D], F32R, tag=f"csr{h}", name=f"csr{h}")
                    nc.scalar.copy(csr, cs_ps[0:1, :])
                    cs_ps2 = psum.tile([P, D], F32, tag="xT", bufs=2, name=f"csb{h}")
                    nc.tensor.matmul(
                        cs_ps2, ones_row[:, :], csr[:, :], start=True, stop=True
                    )
                    nc.scalar.copy(csum_bc[:, h * D : (h + 1) * D], cs_ps2)

            # ---- phase C: per output row tile, software-pipelined so the
            # PE never waits on the DVE epilogue chain: the transpose +
            # output projection of tile i-1 are emitted after the band
            # matmuls of tile i ----
            def tail_stage(x_sb, i):
                xT_psum = psum.tile([P, NC, P], BF16, tag="xT", bufs=2, name="xT_ps")
                for c in range(NC):
                    nc.tensor.transpose(
                        xT_psum[:, c, :], x_sb[:, ts(c, P)], identity_bf
                    )
                xT_sb = work.tile([P, NC, P], BF16, tag="xT_sb")
                nc.scalar.copy(xT_sb, xT_psum)

                out_ps = proj_psum(xT_sb, "wo", 3 if with_biases else None)
                out_sb = outs.tile([P, D], F32, tag="out_sb")
                nc.scalar.copy(out_sb, out_ps)
                if i >= NT - 2:
                    nc.gpsimd.dma_start(out1_d[ts(i, P), 0:P], out_sb[:, 0:P])
                    nc.sync.dma_start(out1_d[ts(i, P), P:D], out_sb[:, P:D])
                else:
                    nc.gpsimd.dma_start(out1_d[ts(i, P), :], out_sb)

            pending = []
            for i in range(NT):
                if i + 3 < NT and i + 3 >= 3:  # q3.. prefetch, 3 tiles ahead
                    qt = qpool.tile([P, NC, P], F16, tag="qT", bufs=4, name="qT_t")
                    nc.sync.dma_start(
                        qt, qP_d[i + 3].rearrange("p (c t) -> p c t", c=NC)
                    )
                    q_tiles.append(qt)
                def band_half(h):
                    ps = psum.tile([P, D], F32, tag="bps", bufs=3, name=f"bps{h}")
                    blocks = _band_blocks(i)
                    for bi, (jl, j) in enumerate(blocks):
                        nc.tensor.matmul(
                            ps,
                            ewm1[:, i, ts(jl, P)],
                            ekk[:, j, h * D : (h + 1) * D],
                            start=(bi == 0),
                            stop=(bi == len(blocks) - 1),
                        )
                    return ps

                # PE order: den matmuls, q proj, num matmuls -- the den
                # eviction + reciprocal start ~1.5us earlier in each tile
                den_ps = band_half(1)
                qT_t = q_tiles[i]
                query_ps = proj_psum(qT_t, "wq", 0 if with_biases else None)
                q_sb = work.tile([P, D], F32, tag="q_sb")
                nc.scalar.activation(q_sb, query_ps, SIGMOID)
                num_ps = band_half(0)
                if i == 0:  # PE runs C(0) above while the DVE/gpsimd csum
                    emit_csum_reduce()  # chains drain; reduce lands here

                den_sb = work.tile([P, D], F32, tag="den_sb", bufs=2)
                nc.vector.tensor_add(den_sb, den_ps, csum_bc[:, D : 2 * D])
                rcp_sb = work.tile([P, D], F32, tag="rcp", bufs=3)
                nc.vector.reciprocal_approx_fast(out=rcp_sb, in_=den_sb)
                numf_sb = work.tile([P, D], F32, tag="numf_sb", bufs=2)
                nc.vector.tensor_add(numf_sb, num_ps, csum_bc[:, 0:D])
                num_sb = outs.tile([P, D], F32, tag="num_sb")
                nc.vector.tensor_mul(num_sb, q_sb, numf_sb)
                if i >= NT - 2:
                    nc.sync.dma_start(num_d[ts(i, P), 0:P], num_sb[:, 0:P])
                    nc.gpsimd.dma_start(num_d[ts(i, P), P:D], num_sb[:, P:D])
                else:
                    nc.sync.dma_start(num_d[ts(i, P), :], num_sb)
                x_sb = work.tile([P, D], BF16, tag="x_sb")
                nc.gpsimd.tensor_mul(x_sb, num_sb, rcp_sb)

                # tail runs 2 tiles behind: the x-chain (DVE evictions +
                # gpsimd mul, ~4us) gets two PE tile-periods of runway
                pending.append((x_sb, i))
                if len(pending) > 2:
                    tail_stage(*pending.pop(0))
            for st in pending:
                tail_stage(*st)

    nc.finalize()
    return nc


def _pack_band(w_bias, local_mask):
    """[128, NT*384] f16: pack[t', i*384 + jl*128 + s'] =
    (exp(w_bias*mask)-1)[i*128+s', (i-1+jl)*128+t']  (transposed blocks)."""
    wbm = np.asarray(w_bias, np.float64) * np.asarray(local_mask, np.float64)
    ewm1 = np.expm1(wbm)
    pack = np.zeros((NT, P, 3 * P), np.float64)
    for i in range(NT):
        for jl, j in _band_blocks(i):
            blk = ewm1[i * P : (i + 1) * P, j * P : (j + 1) * P]
            pack[i, :, jl * P : (jl + 1) * P] = blk.T
    # anything |i-j| >= 2 tiles must be zero for the decomposition to hold
    for i in range(NT):
        lo = max(0, (i - 1) * P)
        hi = min(S, (i + 2) * P)
        row = wbm[i * P : (i + 1) * P]
        if row[:, :lo].any() or row[:, hi:].any():
            raise ValueError("w_bias*mask has support outside the 3-tile band")
    return np.ascontiguousarray(
        pack.transpose(1, 0, 2).reshape(P, NT * 3 * P).astype(np.float16)
    )


def _pack_seq(x):
    """[S, D] -> [NT, 128, NC*128] f16, pack[i,p,c*128+t] = x[i*128+t, c*128+p]."""
    return np.ascontiguousarray(
        x.reshape(NT, P, NC, P)
        .transpose(0, 3, 2, 1)
        .reshape(NT, P, NC * P)
        .astype(np.float16)
    )


def _pack_w(w, dtype="float16"):
    """[D, D] -> [128, NC*512] with pack[p, c*512+n] = w[n, c*128+p]."""
    out = np.ascontiguousarray(
        np.asarray(w, np.float32)
        .T.reshape(NC, P, D)
        .transpose(1, 0, 2)
        .reshape(P, NC * D)
    )
    if dtype == "bfloat16":
        import ml_dtypes

        return out.astype(ml_dtypes.bfloat16)
    return out.astype(np.dtype(dtype))


_CACHE = {}


def _get_nc(with_biases):
    key = bool(with_biases)
    if key not in _CACHE:
        _CACHE[key] = build(key)
    return _CACHE[key]


def run(inputs, trace=False):
    q = np.asarray(inputs["q"], np.float32)
    k = np.asarray(inputs["k"], np.float32)
    v = np.asarray(inputs["v"], np.float32)
    B = q.shape[0]
    assert B == N_CORES and q.shape[1:] == (S, D)

    biases = np.stack(
        [
            np.asarray(inputs["Wq_b"], np.float32),
            np.asarray(inputs["Wk_b"], np.float32),
            np.asarray(inputs["Wv_b"], np.float32),
            np.asarray(inputs["out_b"], np.float32),
        ]
    )
    with_biases = bool(np.any(biases))

    shared = {
        "wqP": _pack_w(inputs["Wq_w"]),
        "wkP": _pack_w(inputs["Wk_w"]),
        "wvP": _pack_w(inputs["Wv_w"]),
        "woP": _pack_w(inputs["out_w"], "bfloat16"),
        "bandP": _pack_band(inputs["w_bias"], inputs["local_mask"]),
    }
    if with_biases:
        shared["biases"] = biases

    in_maps = []
    for b in range(B):
        m = dict(shared)
        kp = _pack_seq(k[b])
        vp = _pack_seq(v[b])
        m["kvP"] = np.ascontiguousarray(np.concatenate([kp, vp], axis=-1))
        m["qP"] = _pack_seq(q[b])
        in_maps.append(m)

    nc = _get_nc(with_biases)
    res = run_bass_kernel_spmd(
        nc, in_maps, core_ids=list(range(N_CORES)), trace=trace
    )
    out1 = np.stack([res.results[b]["out1"] for b in range(B)])
    num = np.stack([res.results[b]["num"] for b in range(B)])
    return (out1, num), res


def kernel(**inputs):
    (out1, num), _ = run(inputs, trace=False)
    return (out1, num)


# revision 28
# speedup vs baseline: 1.0109x; 1.0109x over previous
"""AFT-local attention on 8 Trainium2 NeuronCores.

Reference (per batch element b, S=2048, D=512, window=128):
    query = q @ Wq.T + bq;  Q_ = sigmoid(query)
    key_p = k @ Wk.T + bk;  ek = exp(key_p)
    value = v @ Wv.T + bv;  ekv = ek * value
    ew    = exp(w_bias * local_mask)          # S x S, == 1 outside the band
    num_raw = ew @ ekv ; den = ew @ ek        # dense S x S einsums
    num  = Q_ * num_raw;  x = num / den
    out1 = x @ out_w.T + out_b
    return (out1, num)

Key decomposition: ew = 1 + (exp(wb_masked) - 1) restricted to the band
|i-j| < 128, so  ew @ Y = colsum(Y) + EWM1_band @ Y  where the banded part
only touches <=3 column tiles of 128 per row tile of 128 (46 block matmuls
instead of 256 dense ones).

Sharding: pure data-parallel; batch B=8 -> one batch element per core.

Implementation notes (v2, all rates measured on this silicon):
- PE streams 1 column/cycle at 2.4GHz for EVERY dtype (f16 == bf16 == f32r
  at N=512: ~216ns/MM measured).  PE work is therefore minimized by column
  count, not dtype: ~352 N=512 matmuls + 64 transposes ~= 78us/core floor.
- ewm1 = exp(band)-1 is computed ON HOST and shipped as f16 (values are
  tiny biases |x|<0.04): kills the on-chip ACT exp + DVE add prep of v1.
- ekk ([ekv|ek]) is f16: halves its SBUF/HBM footprint; band matmuls are
  f16; colsum error from f16 rounding averages out over 2048 terms.
- DMA instructions cost ~600-850ns of queue occupancy EACH (size-almost-
  independent), so inputs are shipped as ONE descriptor per [128, >=1kB]
  block: k|v merged per tile, whole weight matrices, the whole band.
  3 queues: sync (most inputs + num out), scalar (2 early loads only),
  gpsimd (out1 + no inputs).
- The S-direction colsum accumulation chain is split DVE/gpsimd by tile
  parity so phase B stays PE-paced.
- Phase C epilogue: den/num PSUM evictions fused with the colsum add on
  DVE, reciprocal_approx_fast on DVE, x = num*rcp on GPSIMD writing bf16;
  transposes + output projection run bf16 (1.0 vs 1.5 cyc/row transpose).
- PSUM evictions (xT, out) on ACT; software-pipelined tail as in v1.
"""

import sys

if "/opt/trn_rl_repo" not in sys.path:
    sys.path.insert(0, "/opt/trn_rl_repo")

import numpy as np

import concourse.bacc as bacc
import concourse.mybir as mybir
import concourse.tile as tile
from concourse.bass import ts

from concourse.bass_utils import run_bass_kernel_spmd
from concourse.masks import make_identity

F32 = mybir.dt.float32
F32R = mybir.dt.float32r
BF16 = mybir.dt.bfloat16
F16 = mybir.dt.float16
EXP = mybir.ActivationFunctionType.Exp
SIGMOID = mybir.ActivationFunctionType.Sigmoid

S = 2048
D = 512
P = 128
NT = S // P  # 16 sequence tiles
NC = D // P  # 4 contraction chunks of the model dim
N_CORES = 8


def _band_blocks(i):
    """Valid (jl, j) column-tile neighbors for row tile i."""
    return [(jl, i - 1 + jl) for jl in range(3) if 0 <= i - 1 + jl < NT]


def build(with_biases):
    nc = bacc.Bacc(None, target_bir_lowering=False, debug=False)

    # host-packed layouts: every [128, ...] DMA partition line is contiguous
    kv_d = nc.dram_tensor("kvP", [NT, P, 2 * NC * P], F16, kind="ExternalInput")
    qP_d = nc.dram_tensor("qP", [NT, P, NC * P], F16, kind="ExternalInput")
    wq_d = nc.dram_tensor("wqP", [P, NC * D], F16, kind="ExternalInput")
    wk_d = nc.dram_tensor("wkP", [P, NC * D], F16, kind="ExternalInput")
    wv_d = nc.dram_tensor("wvP", [P, NC * D], F16, kind="ExternalInput")
    wo_d = nc.dram_tensor("woP", [P, NC * D], BF16, kind="ExternalInput")
    band_d = nc.dram_tensor("bandP", [P, NT * 3 * P], F16, kind="ExternalInput")
    if with_biases:
        # rows: bq, bk, bv, bo
        bias_d = nc.dram_tensor("biases", [4, D], F32, kind="ExternalInput")
    out1_d = nc.dram_tensor("out1", [S, D], F32, kind="ExternalOutput")
    num_d = nc.dram_tensor("num", [S, D], F32, kind="ExternalOutput")

    with tile.TileContext(nc) as tc:
        with (
            tc.tile_pool(name="consts", bufs=1) as consts,
            tc.tile_pool(name="weights", bufs=1) as wpool,
            tc.tile_pool(name="ekk_pool", bufs=1) as ekkpool,
            tc.tile_pool(name="kv_in", bufs=1) as kvpool,
            tc.tile_pool(name="q_in", bufs=1) as qpool,
            tc.tile_pool(name="work", bufs=3) as work,
            tc.tile_pool(name="outs", bufs=3) as outs,
            tc.tile_pool(name="psum", bufs=1, space="PSUM") as psum,
        ):
            # ---- input staging.  Emission order = per-queue issue order, so
            # the first matmul's operands (kv tile 0, wk chunk 0) lead their
            # queues: wk on sync, kv0 on scalar ----
            w_sb = {}
            kv_tiles = []

            def emit_kv(j, eng, split=False):
                t = kvpool.tile([P, 2, NC, P], F16, tag="kv", bufs=6, name="kv_t")
                src = kv_d[j].rearrange("p (u c t) -> p u c t", u=2, c=NC)
                if split:  # k-half lands first so the lead matmul starts early
                    eng.dma_start(t[:, 0], src[:, 0])
                    eng.dma_start(t[:, 1], src[:, 1])
                else:
                    eng.dma_start(t, src)
                kv_tiles.append(t)

            # critical-path order on the single input queue: kv0 k-half +
            # wk chunk 0 feed the first matmul; v0/wv follow ~1.7us later
            wk_t = wpool.tile([P, NC, D], F16, tag="w_wk", name="w_wk")
            wk_src = wk_d[:, :].rearrange("p (c n) -> p c n", c=NC)
            kv0 = kvpool.tile([P, 2, NC, P], F16, tag="kv", bufs=6, name="kv_t")
            kv0_src = kv_d[0].rearrange("p (u c t) -> p u c t", u=2, c=NC)
            nc.sync.dma_start(kv0[:, 0], kv0_src[:, 0])
            nc.sync.dma_start(wk_t[:, 0, :], wk_src[:, 0, :])
            wv_t = wpool.tile([P, NC, D], F16, tag="w_wv", name="w_wv")
            wv_src = wv_d[:, :].rearrange("p (c n) -> p c n", c=NC)
            nc.sync.dma_start(kv0[:, 1], kv0_src[:, 1])
            nc.sync.dma_start(wv_t[:, 0, :], wv_src[:, 0, :])
            nc.sync.dma_start(wk_t[:, 1:, :], wk_src[:, 1:, :])
            nc.sync.dma_start(wv_t[:, 1:, :], wv_src[:, 1:, :])
            w_sb["wk"] = wk_t
            w_sb["wv"] = wv_t
            kv_tiles.append(kv0)

            emit_kv(1, nc.sync, split=True)
            emit_kv(2, nc.sync)

            # whole banded exp(w_bias)-1, host-precomputed, one descriptor
            ewm1 = wpool.tile([P, NT, 3 * P], F16, tag="ewm1", name="ewm1")
            nc.sync.dma_start(
                ewm1, band_d[:, :].rearrange("p (i f) -> p i f", i=NT)
            )

            wq_t = wpool.tile([P, NC, D], F16, tag="w_wq", name="w_wq")
            nc.sync.dma_start(
                wq_t, wq_d[:, :].rearrange("p (c n) -> p c n", c=NC)
            )
            w_sb["wq"] = wq_t
            wo_t = wpool.tile([P, NC, D], BF16, tag="w_wo", name="w_wo")
            nc.sync.dma_start(
                wo_t, wo_d[:, :].rearrange("p (c n) -> p c n", c=NC)
            )
            w_sb["wo"] = wo_t
            if with_biases:
                bias_sb = consts.tile([4, D], F32R)
                nc.sync.dma_start(bias_sb, bias_d[:, :].bitcast(F32R))

            # ---- constants (gpsimd + vector lead with these; neither queue
            # has early DMA duties in this schedule) ----
            identity_f32 = consts.tile([P, P], F32)
            make_identity(nc, identity_f32)
            identity_bf = consts.tile([P, P], BF16)
            nc.vector.tensor_copy(identity_bf, identity_f32)
            ones_f32 = consts.tile([P, 1], F32)
            nc.gpsimd.memset(ones_f32, 1.0)
            ones_col = consts.tile([P, 1], F32R)  # lhsT for column sums
            nc.vector.tensor_copy(ones_col, ones_f32)
            ones_row = consts.tile([1, P], F32R)  # lhsT for partition bcast
            nc.vector.tensor_copy(ones_row, ones_f32[0:1, 0:1].broadcast_to([1, P]))

            # ---- ekk: per seq-tile j, [ekv | ek] along free dim, f16 ----
            ekk = ekkpool.tile([P, NT, 2 * D], F16)

            def proj_psum(xT_tile, wname, bias_row):
                """psum [128, 512] = (x @ W.T + b) for one 128-seq tile."""
                # bufs=4: the k-proj of tile j must not wait on the DVE ekv
                # eviction chain, which trails the PE by ~1.5 tiles
                ps = psum.tile([P, D], F32, tag="ps", bufs=4, name="proj_ps")
                for c in range(NC):
                    nc.tensor.matmul(
                        ps,
                        xT_tile[:, c, :],
                        w_sb[wname][:, c, :],
                        start=(c == 0),
                        stop=(c == NC - 1 and bias_row is None),
                    )
                if bias_row is not None:
                    nc.tensor.matmul(
                        ps,
                        ones_row[:, :],
                        bias_sb[bias_row : bias_row + 1, :],
                        start=False,
                        stop=True,
                    )
                return ps

            # ---- phase B: ek / ekv + column-sum chain (DVE/gpsimd split) ----
            # single accumulator, split by COLUMN range: gpsimd sums the
            # first 640 columns, DVE the last 384, every tile -- a flat
            # ~1.4us/tile on each engine instead of 2.3us spikes by parity
            acc = wpool.tile([P, 2 * D], F32R, tag="acc", name="acc")
            nc.gpsimd.memset(acc.bitcast(F32), 0.0)
            CSPLIT = 640
            q_tiles = []
            for j in range(NT):
                if j + 3 < NT:
                    emit_kv(j + 3, nc.sync)
                kvt = kv_tiles[j]

                keyp_ps = proj_psum(kvt[:, 0], "wk", 1 if with_biases else None)
                ek_view = ekk[:, j, D : 2 * D]
                nc.scalar.activation(ek_view, keyp_ps, EXP)

                val_ps = proj_psum(kvt[:, 1], "wv", 2 if with_biases else None)
                nc.vector.tensor_mul(ekk[:, j, 0:D], ek_view, val_ps)

                # column-sum accumulation, column-split gpsimd/DVE
                src = ekk[:, j, :]
                nc.gpsimd.tensor_add(
                    acc[:, 0:CSPLIT], acc[:, 0:CSPLIT], src[:, 0:CSPLIT]
                )
                nc.vector.tensor_add(
                    acc[:, CSPLIT:], acc[:, CSPLIT:], src[:, CSPLIT:]
                )

                if j >= NT - 3:  # q0..q2 prefetch at tail of phase B
                    i0 = j - (NT - 3)
                    qt = qpool.tile([P, NC, P], F16, tag="qT", bufs=4, name="qT_t")
                    nc.sync.dma_start(
                        qt, qP_d[i0].rearrange("p (c t) -> p c t", c=NC)
                    )
                    q_tiles.append(qt)

            # partition-reduce on the PE (ones^T @ acc), evict on ACT, then
            # broadcast back over 128 partitions via a K=1 matmul
            csum_bc = wpool.tile([P, 2 * D], F32, tag="csum_bc", name="csum_bc")

            def emit_csum_reduce():
                # both partial accs reduced straight into one PSUM tile --
                # no DVE combine on the B->C critical path
                for h in range(2):
                    cs_ps = psum.tile([P, D], F32, tag="ps", bufs=4, name=f"cs{h}")
                    nc.tensor.matmul(
                        cs_ps[0:1, :],
                        ones_col[:, :],
                        acc[:, h * D : (h + 1) * D],
                        start=True,
                        stop=True,
                    )
                    csr = consts.tile([1, D], F32R, tag=f"csr{h}", name=f"csr{h}")
                    nc.scalar.copy(csr, cs_ps[0:1, :])
                    cs_ps2 = psum.tile([P, D], F32, tag="ps", bufs=4, name=f"csb{h}")
                    nc.tensor.matmul(
                        cs_ps2, ones_row[:, :], csr[:, :], start=True, stop=True
                    )
                    nc.scalar.copy(csum_bc[:, h * D : (h + 1) * D], cs_ps2)

            # ---- phase C: per output row tile, software-pipelined so the
            # PE never waits on the DVE epilogue chain: the transpose +
            # output projection of tile i-1 are emitted after the band
            # matmuls of tile i ----
            def tail_stage(x_sb, i):
                xT_psum = psum.tile([P, NC, P], BF16, tag="xT", bufs=1, name="xT_ps")
                for c in range(NC):
                    nc.tensor.transpose(
                        xT_psum[:, c, :], x_sb[:, ts(c, P)], identity_bf
                    )
                xT_sb = work.tile([P, NC, P], BF16, tag="xT_sb")
                nc.scalar.copy(xT_sb, xT_psum)

                out_ps = proj_psum(xT_sb, "wo", 3 if with_biases else None)
                out_sb = outs.tile([P, D], F32, tag="out_sb")
                nc.scalar.copy(out_sb, out_ps)
                if i >= NT - 2:
                    nc.gpsimd.dma_start(out1_d[ts(i, P), 0:P], out_sb[:, 0:P])
                    nc.sync.dma_start(out1_d[ts(i, P), P:D], out_sb[:, P:D])
                else:
                    nc.gpsimd.dma_start(out1_d[ts(i, P), :], out_sb)

            pending = []
            for i in range(NT):
                if i + 3 < NT and i + 3 >= 3:  # q3.. prefetch, 3 tiles ahead
                    qt = qpool.tile([P, NC, P], F16, tag="qT", bufs=4, name="qT_t")
                    nc.sync.dma_start(
                        qt, qP_d[i + 3].rearrange("p (c t) -> p c t", c=NC)
                    )
                    q_tiles.append(qt)
                def band_half(h):
                    ps = psum.tile([P, D], F32, tag="bps", bufs=3, name=f"bps{h}")
                    blocks = _band_blocks(i)
                    for bi, (jl, j) in enumerate(blocks):
                        nc.tensor.matmul(
                            ps,
                            ewm1[:, i, ts(jl, P)],
                            ekk[:, j, h * D : (h + 1) * D],
                            start=(bi == 0),
                            stop=(bi == len(blocks) - 1),
                        )
                    return ps

                # PE order: den matmuls, q proj, num matmuls -- the den
                # eviction + reciprocal start ~1.5us earlier in each tile
                den_ps = band_half(1)
                qT_t = q_tiles[i]
                query_ps = proj_psum(qT_t, "wq", 0 if with_biases else None)
                q_sb = work.tile([P, D], F32, tag="q_sb")
                nc.scalar.activation(q_sb, query_ps, SIGMOID)
                num_ps = band_half(0)
                if i == 0:  # PE runs C(0) above while the DVE/gpsimd csum
                    emit_csum_reduce()  # chains drain; reduce lands here

                den_sb = work.tile([P, D], F32, tag="den_sb", bufs=2)
                nc.vector.tensor_add(den_sb, den_ps, csum_bc[:, D : 2 * D])
                rcp_sb = work.tile([P, D], F32, tag="rcp", bufs=3)
                nc.vector.reciprocal_approx_fast(out=rcp_sb, in_=den_sb)
                numf_sb = work.tile([P, D], F32, tag="numf_sb", bufs=2)
                nc.vector.tensor_add(numf_sb, num_ps, csum_bc[:, 0:D])
                num_sb = outs.tile([P, D], F32, tag="num_sb")
                nc.vector.tensor_mul(num_sb, q_sb, numf_sb)
                if i >= NT - 2:
                    nc.sync.dma_start(num_d[ts(i, P), 0:P], num_sb[:, 0:P])
                    nc.gpsimd.dma_start(num_d[ts(i, P), P:D], num_sb[:, P:D])
                else:
                    nc.sync.dma_start(num_d[ts(i, P), :], num_sb)
                x_sb = work.tile([P, D], BF16, tag="x_sb")
                nc.gpsimd.tensor_mul(x_sb, num_sb, rcp_sb)

                # tail runs 2 tiles behind: the x-chain (DVE evictions +
                # gpsimd mul, ~4us) gets two PE tile-periods of runway
                pending.append((x_sb, i))
                if len(pending) > 2:
                    tail_stage(*pending.pop(0))
            for st in pending:
                tail_stage(*st)

    nc.finalize()
    return nc


def _pack_band(w_bias, local_mask):
    """[128, NT*384] f16: pack[t', i*384 + jl*128 + s'] =
    (exp(w_bias*mask)-1)[i*128+s', (i-1+jl)*128+t']  (transposed blocks)."""
    wbm = np.asarray(w_bias, np.float64) * np.asarray(local_mask, np.float64)
    ewm1 = np.expm1(wbm)
    pack = np.zeros((NT, P, 3 * P), np.float64)
    for i in range(NT):
        for jl, j in _band_blocks(i):
            blk = ewm1[i * P : (i + 1) * P, j * P : (j + 1) * P]
            pack[i, :, jl * P : (jl + 1) * P] = blk.T
    # anything |i-j| >= 2 tiles must be zero for the decomposition to hold
    for i in range(NT):
        lo = max(0, (i - 1) * P)
        hi = min(S, (i + 2) * P)
        row = wbm[i * P : (i + 1) * P]
        if row[:, :lo].any() or row[:, hi:].any():
            raise ValueError("w_bias*mask has support outside the 3-tile band")
    return np.ascontiguousarray(
        pack.transpose(1, 0, 2).reshape(P, NT * 3 * P).astype(np.float16)
    )


def _pack_seq(x):
    """[S, D] -> [NT, 128, NC*128] f16, pack[i,p,c*128+t] = x[i*128+t, c*128+p]."""
    return np.ascontiguousarray(
        x.reshape(NT, P, NC, P)
        .transpose(0, 3, 2, 1)
        .reshape(NT, P, NC * P)
        .astype(np.float16)
    )


def _pack_w(w, dtype="float16"):
    """[D, D] -> [128, NC*512] with pack[p, c*512+n] = w[n, c*128+p]."""
    out = np.ascontiguousarray(
        np.asarray(w, np.float32)
        .T.reshape(NC, P, D)
        .transpose(1, 0, 2)
        .reshape(P, NC * D)
    )
    if dtype == "bfloat16":
        import ml_dtypes

        return out.astype(ml_dtypes.bfloat16)
    return out.astype(np.dtype(dtype))


_CACHE = {}


def _get_nc(with_biases):
    key = bool(with_biases)
    if key not in _CACHE:
        _CACHE[key] = build(key)
    return _CACHE[key]


def run(inputs, trace=False):
    q = np.asarray(inputs["q"], np.float32)
    k = np.asarray(inputs["k"], np.float32)
    v = np.asarray(inputs["v"], np.float32)
    B = q.shape[0]
    assert B == N_CORES and q.shape[1:] == (S, D)

    biases = np.stack(
        [
            np.asarray(inputs["Wq_b"], np.float32),
            np.asarray(inputs["Wk_b"], np.float32),
            np.asarray(inputs["Wv_b"], np.float32),
            np.asarray(inputs["out_b"], np.float32),
        ]
    )
    with_biases = bool(np.any(biases))

    shared = {
        "wqP": _pack_w(inputs["Wq_w"]),
        "wkP": _pack_w(inputs["Wk_w"]),
        "wvP": _pack_w(inputs["Wv_w"]),
        "woP": _pack_w(inputs["out_w"], "bfloat16"),
        "bandP": _pack_band(inputs["w_bias"], inputs["local_mask"]),
    }
    if with_biases:
        shared["biases"] = biases

    in_maps = []
    for b in range(B):
        m = dict(shared)
        kp = _pack_seq(k[b])
        vp = _pack_seq(v[b])
        m["kvP"] = np.ascontiguousarray(np.concatenate([kp, vp], axis=-1))
        m["qP"] = _pack_seq(q[b])
        in_maps.append(m)

    nc = _get_nc(with_biases)
    res = run_bass_kernel_spmd(
        nc, in_maps, core_ids=list(range(N_CORES)), trace=trace
    )
    out1 = np.stack([res.results[b]["out1"] for b in range(B)])
    num = np.stack([res.results[b]["num"] for b in range(B)])
    return (out1, num), res


def kernel(**inputs):
    (out1, num), _ = run(inputs, trace=False)
    return (out1, num)


# revision 33
# speedup vs baseline: 1.0508x; 1.0395x over previous
"""AFT-local attention on 8 Trainium2 NeuronCores.

Reference (per batch element b, S=2048, D=512, window=128):
    query = q @ Wq.T + bq;  Q_ = sigmoid(query)
    key_p = k @ Wk.T + bk;  ek = exp(key_p)
    value = v @ Wv.T + bv;  ekv = ek * value
    ew    = exp(w_bias * local_mask)          # S x S, == 1 outside the band
    num_raw = ew @ ekv ; den = ew @ ek        # dense S x S einsums
    num  = Q_ * num_raw;  x = num / den
    out1 = x @ out_w.T + out_b
    return (out1, num)

Key decomposition: ew = 1 + (exp(wb_masked) - 1) restricted to the band
|i-j| < 128, so  ew @ Y = colsum(Y) + EWM1_band @ Y  where the banded part
only touches <=3 column tiles of 128 per row tile of 128 (46 block matmuls
instead of 256 dense ones).

Sharding: pure data-parallel; batch B=8 -> one batch element per core.

Implementation notes (v2, all rates measured on this silicon):
- PE streams 1 column/cycle at 2.4GHz for EVERY dtype (f16 == bf16 == f32r
  at N=512: ~216ns/MM measured).  PE work is therefore minimized by column
  count, not dtype: ~352 N=512 matmuls + 64 transposes ~= 78us/core floor.
- ewm1 = exp(band)-1 is computed ON HOST and shipped as f16 (values are
  tiny biases |x|<0.04): kills the on-chip ACT exp + DVE add prep of v1.
- ekk ([ekv|ek]) is f16: halves its SBUF/HBM footprint; band matmuls are
  f16; colsum error from f16 rounding averages out over 2048 terms.
- DMA instructions cost ~600-850ns of queue occupancy EACH (size-almost-
  independent), so inputs are shipped as ONE descriptor per [128, >=1kB]
  block: k|v merged per tile, whole weight matrices, the whole band.
  3 queues: sync (most inputs + num out), scalar (2 early loads only),
  gpsimd (out1 + no inputs).
- The S-direction colsum accumulation chain is split DVE/gpsimd by tile
  parity so phase B stays PE-paced.
- Phase C epilogue: den/num PSUM evictions fused with the colsum add on
  DVE, reciprocal_approx_fast on DVE, x = num*rcp on GPSIMD writing bf16;
  transposes + output projection run bf16 (1.0 vs 1.5 cyc/row transpose).
- PSUM evictions (xT, out) on ACT; software-pipelined tail as in v1.
"""

import sys

if "/opt/trn_rl_repo" not in sys.path:
    sys.path.insert(0, "/opt/trn_rl_repo")

import numpy as np

import concourse.bacc as bacc
import concourse.mybir as mybir
import concourse.tile as tile
from concourse.bass import ts

from concourse.bass_utils import run_bass_kernel_spmd
from concourse.masks import make_identity

F32 = mybir.dt.float32
F32R = mybir.dt.float32r
BF16 = mybir.dt.bfloat16
F16 = mybir.dt.float16
EXP = mybir.ActivationFunctionType.Exp
SIGMOID = mybir.ActivationFunctionType.Sigmoid

S = 2048
D = 512
P = 128
NT = S // P  # 16 sequence tiles
NC = D // P  # 4 contraction chunks of the model dim
N_CORES = 8


def _band_blocks(i):
    """Valid (jl, j) column-tile neighbors for row tile i."""
    return [(jl, i - 1 + jl) for jl in range(3) if 0 <= i - 1 + jl < NT]


def build(with_biases):
    nc = bacc.Bacc(None, target_bir_lowering=False, debug=False)

    # host-packed layouts: every [128, ...] DMA partition line is contiguous
    kv_d = nc.dram_tensor("kvP", [NT, P, 2 * NC * P], F16, kind="ExternalInput")
    qP_d = nc.dram_tensor("qP", [NT, P, NC * P], F16, kind="ExternalInput")
    wq_d = nc.dram_tensor("wqP", [P, NC * D], F16, kind="ExternalInput")
    wk_d = nc.dram_tensor("wkP", [P, NC * D], F16, kind="ExternalInput")
    wv_d = nc.dram_tensor("wvP", [P, NC * D], F16, kind="ExternalInput")
    wo_d = nc.dram_tensor("woP", [P, NC * D], BF16, kind="ExternalInput")
    band_d = nc.dram_tensor("bandP", [P, NT * 3 * P], F16, kind="ExternalInput")
    if with_biases:
        # rows: bq, bk, bv, bo
        bias_d = nc.dram_tensor("biases", [4, D], F32, kind="ExternalInput")
    out1_d = nc.dram_tensor("out1", [S, D], F32, kind="ExternalOutput")
    num_d = nc.dram_tensor("num", [S, D], F32, kind="ExternalOutput")

    with tile.TileContext(nc) as tc:
        with (
            tc.tile_pool(name="consts", bufs=1) as consts,
            tc.tile_pool(name="weights", bufs=1) as wpool,
            tc.tile_pool(name="ekk_pool", bufs=1) as ekkpool,
            tc.tile_pool(name="kv_in", bufs=1) as kvpool,
            tc.tile_pool(name="q_in", bufs=1) as qpool,
            tc.tile_pool(name="work", bufs=3) as work,
            tc.tile_pool(name="outs", bufs=3) as outs,
            tc.tile_pool(name="psum", bufs=1, space="PSUM") as psum,
        ):
            # ---- input staging.  Emission order = per-queue issue order, so
            # the first matmul's operands (kv tile 0, wk chunk 0) lead their
            # queues: wk on sync, kv0 on scalar ----
            w_sb = {}
            kv_tiles = []

            def emit_kv(j, eng, split=False):
                t = kvpool.tile([P, 2, NC, P], F16, tag="kv", bufs=6, name="kv_t")
                src = kv_d[j].rearrange("p (u c t) -> p u c t", u=2, c=NC)
                if split:  # k-half lands first so the lead matmul starts early
                    eng.dma_start(t[:, 0], src[:, 0])
                    eng.dma_start(t[:, 1], src[:, 1])
                else:
                    eng.dma_start(t, src)
                kv_tiles.append(t)

            # critical-path order on the single input queue: kv0 k-half +
            # wk chunk 0 feed the first matmul; v0/wv follow ~1.7us later
            wk_t = wpool.tile([P, NC, D], F16, tag="w_wk", name="w_wk")
            wk_src = wk_d[:, :].rearrange("p (c n) -> p c n", c=NC)
            kv0 = kvpool.tile([P, 2, NC, P], F16, tag="kv", bufs=6, name="kv_t")
            kv0_src = kv_d[0].rearrange("p (u c t) -> p u c t", u=2, c=NC)
            nc.sync.dma_start(kv0[:, 0], kv0_src[:, 0])
            nc.sync.dma_start(wk_t[:, 0, :], wk_src[:, 0, :])
            # scalar's HWDGE queue is free after its ACT-table preamble:
            # it carries the v-side and phase-C inputs in parallel with sync
            wv_t = wpool.tile([P, NC, D], F16, tag="w_wv", name="w_wv")
            wv_src = wv_d[:, :].rearrange("p (c n) -> p c n", c=NC)
            nc.sync.dma_start(kv0[:, 1], kv0_src[:, 1])
            nc.scalar.dma_start(wv_t[:, 0, :], wv_src[:, 0, :])
            nc.sync.dma_start(wk_t[:, 1:, :], wk_src[:, 1:, :])
            nc.scalar.dma_start(wv_t[:, 1:, :], wv_src[:, 1:, :])
            w_sb["wk"] = wk_t
            w_sb["wv"] = wv_t
            kv_tiles.append(kv0)

            emit_kv(1, nc.sync, split=True)
            emit_kv(2, nc.scalar)

            # whole banded exp(w_bias)-1, host-precomputed, one descriptor
            ewm1 = wpool.tile([P, NT, 3 * P], F16, tag="ewm1", name="ewm1")
            nc.scalar.dma_start(
                ewm1, band_d[:, :].rearrange("p (i f) -> p i f", i=NT)
            )

            wq_t = wpool.tile([P, NC, D], F16, tag="w_wq", name="w_wq")
            nc.scalar.dma_start(
                wq_t, wq_d[:, :].rearrange("p (c n) -> p c n", c=NC)
            )
            w_sb["wq"] = wq_t
            wo_t = wpool.tile([P, NC, D], BF16, tag="w_wo", name="w_wo")
            nc.scalar.dma_start(
                wo_t, wo_d[:, :].rearrange("p (c n) -> p c n", c=NC)
            )
            w_sb["wo"] = wo_t
            if with_biases:
                bias_sb = consts.tile([4, D], F32R)
                nc.sync.dma_start(bias_sb, bias_d[:, :].bitcast(F32R))

            # ---- constants (gpsimd + vector lead with these; neither queue
            # has early DMA duties in this schedule) ----
            identity_f32 = consts.tile([P, P], F32)
            make_identity(nc, identity_f32)
            identity_bf = consts.tile([P, P], BF16)
            nc.vector.tensor_copy(identity_bf, identity_f32)
            ones_f32 = consts.tile([P, 1], F32)
            nc.gpsimd.memset(ones_f32, 1.0)
            ones_col = consts.tile([P, 1], F32R)  # lhsT for column sums
            nc.vector.tensor_copy(ones_col, ones_f32)
            ones_row = consts.tile([1, P], F32R)  # lhsT for partition bcast
            nc.vector.tensor_copy(ones_row, ones_f32[0:1, 0:1].broadcast_to([1, P]))
            ones_col16 = consts.tile([P, 1], F16)  # lhsT for tile-15 colsum
            nc.vector.tensor_copy(ones_col16, ones_f32)

            # ---- ekk: per seq-tile j, [ekv | ek] along free dim, f16 ----
            ekk = ekkpool.tile([P, NT, 2 * D], F16)

            def proj_psum(xT_tile, wname, bias_row):
                """psum [128, 512] = (x @ W.T + b) for one 128-seq tile."""
                # bufs=4: the k-proj of tile j must not wait on the DVE ekv
                # eviction chain, which trails the PE by ~1.5 tiles
                ps = psum.tile([P, D], F32, tag="ps", bufs=4, name="proj_ps")
                for c in range(NC):
                    nc.tensor.matmul(
                        ps,
                        xT_tile[:, c, :],
                        w_sb[wname][:, c, :],
                        start=(c == 0),
                        stop=(c == NC - 1 and bias_row is None),
                    )
                if bias_row is not None:
                    nc.tensor.matmul(
                        ps,
                        ones_row[:, :],
                        bias_sb[bias_row : bias_row + 1, :],
                        start=False,
                        stop=True,
                    )
                return ps

            # ---- phase B: ek / ekv + column-sum chain (DVE/gpsimd split) ----
            # single accumulator, split by COLUMN range: gpsimd sums the
            # first 640 columns, DVE the last 384, every tile -- a flat
            # ~1.4us/tile on each engine instead of 2.3us spikes by parity
            acc = wpool.tile([P, 2 * D], F32R, tag="acc", name="acc")
            nc.gpsimd.memset(acc.bitcast(F32), 0.0)
            CSPLIT = 640
            q_tiles = []
            for j in range(NT):
                if j + 3 < NT:
                    emit_kv(j + 3, nc.sync)
                kvt = kv_tiles[j]

                keyp_ps = proj_psum(kvt[:, 0], "wk", 1 if with_biases else None)
                ek_view = ekk[:, j, D : 2 * D]
                nc.scalar.activation(ek_view, keyp_ps, EXP)

                val_ps = proj_psum(kvt[:, 1], "wv", 2 if with_biases else None)
                nc.vector.tensor_mul(ekk[:, j, 0:D], ek_view, val_ps)

                # column-sum accumulation, column-split gpsimd/DVE; the last
                # tile goes straight into the PE reduce (emit_csum_reduce) so
                # the B->C transition doesn't wait a full accumulate chain
                if j < NT - 1:
                    src = ekk[:, j, :]
                    nc.gpsimd.tensor_add(
                        acc[:, 0:CSPLIT], acc[:, 0:CSPLIT], src[:, 0:CSPLIT]
                    )
                    nc.vector.tensor_add(
                        acc[:, CSPLIT:], acc[:, CSPLIT:], src[:, CSPLIT:]
                    )

                if j >= NT - 3:  # q0..q2 prefetch at tail of phase B
                    i0 = j - (NT - 3)
                    qt = qpool.tile([P, NC, P], F16, tag="qT", bufs=4, name="qT_t")
                    nc.sync.dma_start(
                        qt, qP_d[i0].rearrange("p (c t) -> p c t", c=NC)
                    )
                    q_tiles.append(qt)

            # partition-reduce on the PE (ones^T @ acc), evict on ACT, then
            # broadcast back over 128 partitions via a K=1 matmul
            csum_bc = wpool.tile([P, 2 * D], F32, tag="csum_bc", name="csum_bc")

            def emit_csum_reduce():
                # both partial accs reduced straight into one PSUM tile --
                # no DVE combine on the B->C critical path
                # den half (h=1) first: the phase-C epilogue needs it first
                cs_ps = {}
                for h in (1, 0):
                    cs_ps[h] = psum.tile(
                        [P, D], F32, tag="ps", bufs=4, name=f"cs{h}"
                    )
                    nc.tensor.matmul(
                        cs_ps[h][0:1, :],
                        ones_col[:, :],
                        acc[:, h * D : (h + 1) * D],
                        start=True,
                        stop=False,
                    )
                    nc.tensor.matmul(  # tile 15 joins here, skipping the
                        cs_ps[h][0:1, :],  # DVE/gpsimd accumulate chain
                        ones_col16[:, :],
                        ekk[:, NT - 1, h * D : (h + 1) * D],
                        start=False,
                        stop=True,
                    )
                for h in (1, 0):
                    csr = consts.tile([1, D], F32R, tag=f"csr{h}", name=f"csr{h}")
                    nc.scalar.copy(csr, cs_ps[h][0:1, :])
                    cs_ps2 = psum.tile([P, D], F32, tag="ps", bufs=4, name=f"csb{h}")
                    nc.tensor.matmul(
                        cs_ps2, ones_row[:, :], csr[:, :], start=True, stop=True
                    )
                    nc.scalar.copy(csum_bc[:, h * D : (h + 1) * D], cs_ps2)

            # ---- phase C: per output row tile, software-pipelined so the
            # PE never waits on the DVE epilogue chain: the transpose +
            # output projection of tile i-1 are emitted after the band
            # matmuls of tile i ----
            def tail_stage(x_sb, i):
                xT_psum = psum.tile([P, NC, P], BF16, tag="xT", bufs=1, name="xT_ps")
                for c in range(NC):
                    nc.tensor.transpose(
                        xT_psum[:, c, :], x_sb[:, ts(c, P)], identity_bf
                    )
                xT_sb = work.tile([P, NC, P], BF16, tag="xT_sb")
                nc.scalar.copy(xT_sb, xT_psum)

                out_ps = proj_psum(xT_sb, "wo", 3 if with_biases else None)
                out_sb = outs.tile([P, D], F32, tag="out_sb")
                nc.scalar.copy(out_sb, out_ps)
                if i >= NT - 2:
                    nc.gpsimd.dma_start(out1_d[ts(i, P), 0:P], out_sb[:, 0:P])
                    nc.sync.dma_start(out1_d[ts(i, P), P:D], out_sb[:, P:D])
                else:
                    nc.gpsimd.dma_start(out1_d[ts(i, P), :], out_sb)

            pending = []
            for i in range(NT):
                if i + 3 < NT and i + 3 >= 3:  # q3.. prefetch, 3 tiles ahead
                    qt = qpool.tile([P, NC, P], F16, tag="qT", bufs=4, name="qT_t")
                    nc.sync.dma_start(
                        qt, qP_d[i + 3].rearrange("p (c t) -> p c t", c=NC)
                    )
                    q_tiles.append(qt)
                def band_half(h):
                    ps = psum.tile([P, D], F32, tag="bps", bufs=3, name=f"bps{h}")
                    blocks = _band_blocks(i)
                    for bi, (jl, j) in enumerate(blocks):
                        nc.tensor.matmul(
                            ps,
                            ewm1[:, i, ts(jl, P)],
                            ekk[:, j, h * D : (h + 1) * D],
                            start=(bi == 0),
                            stop=(bi == len(blocks) - 1),
                        )
                    return ps

                # PE order: den matmuls, q proj, num matmuls -- the den
                # eviction + reciprocal start ~1.5us earlier in each tile
                den_ps = band_half(1)
                qT_t = q_tiles[i]
                query_ps = proj_psum(qT_t, "wq", 0 if with_biases else None)
                q_sb = work.tile([P, D], F32, tag="q_sb")
                nc.scalar.activation(q_sb, query_ps, SIGMOID)
                num_ps = band_half(0)
                if i == 0:  # PE runs C(0) above while the DVE/gpsimd csum
                    emit_csum_reduce()  # chains drain; reduce lands here

                den_sb = work.tile([P, D], F32, tag="den_sb", bufs=2)
                nc.vector.tensor_add(den_sb, den_ps, csum_bc[:, D : 2 * D])
                rcp_sb = work.tile([P, D], F32, tag="rcp", bufs=3)
                nc.vector.reciprocal_approx_fast(out=rcp_sb, in_=den_sb)
                numf_sb = work.tile([P, D], F32, tag="numf_sb", bufs=2)
                nc.vector.tensor_add(numf_sb, num_ps, csum_bc[:, 0:D])
                num_sb = outs.tile([P, D], F32, tag="num_sb")
                nc.vector.tensor_mul(num_sb, q_sb, numf_sb)
                if i >= NT - 2:
                    nc.sync.dma_start(num_d[ts(i, P), 0:P], num_sb[:, 0:P])
                    nc.gpsimd.dma_start(num_d[ts(i, P), P:D], num_sb[:, P:D])
                else:
                    nc.sync.dma_start(num_d[ts(i, P), :], num_sb)
                x_sb = work.tile([P, D], BF16, tag="x_sb")
                # final tiles: DVE is draining and ~2x faster than gpsimd,
                # and the x-chain is the whole critical path at the end
                xeng = nc.vector if i >= NT - 2 else nc.gpsimd
                xeng.tensor_mul(x_sb, num_sb, rcp_sb)

                # tail runs 2 tiles behind: the x-chain (DVE evictions +
                # gpsimd mul, ~4us) gets two PE tile-periods of runway
                pending.append((x_sb, i))
                if len(pending) > 2:
                    tail_stage(*pending.pop(0))
            for st in pending:
                tail_stage(*st)

    nc.finalize()
    return nc


def _pack_band(w_bias, local_mask):
    """[128, NT*384] f16: pack[t', i*384 + jl*128 + s'] =
    (exp(w_bias*mask)-1)[i*128+s', (i-1+jl)*128+t']  (transposed blocks)."""
    wbm = np.asarray(w_bias, np.float64) * np.asarray(local_mask, np.float64)
    ewm1 = np.expm1(wbm)
    pack = np.zeros((NT, P, 3 * P), np.float64)
    for i in range(NT):
        for jl, j in _band_blocks(i):
            blk = ewm1[i * P : (i + 1) * P, j * P : (j + 1) * P]
            pack[i, :, jl * P : (jl + 1) * P] = blk.T
    # anything |i-j| >= 2 tiles must be zero for the decomposition to hold
    for i in range(NT):
        lo = max(0, (i - 1) * P)
        hi = min(S, (i + 2) * P)
        row = wbm[i * P : (i + 1) * P]
        if row[:, :lo].any() or row[:, hi:].any():
            raise ValueError("w_bias*mask has support outside the 3-tile band")
    return np.ascontiguousarray(
        pack.transpose(1, 0, 2).reshape(P, NT * 3 * P).astype(np.float16)
    )


def _pack_seq(x):
    """[S, D] -> [NT, 128, NC*128] f16, pack[i,p,c*128+t] = x[i*128+t, c*128+p]."""
    return np.ascontiguousarray(
        x.reshape(NT, P, NC, P)
        .transpose(0, 3, 2, 1)
        .reshape(NT, P, NC * P)
        .astype(np.float16)
    )


def _pack_w(w, dtype="float16"):
    """[D, D] -> [128, NC*512] with pack[p, c*512+n] = w[n, c*128+p]."""
    out = np.ascontiguousarray(
        np.asarray(w, np.float32)
        .T.reshape(NC, P, D)
        .transpose(1, 0, 2)
        .reshape(P, NC * D)
    )
    if dtype == "bfloat16":
        import ml_dtypes

        return out.astype(ml_dtypes.bfloat16)
    return out.astype(np.dtype(dtype))


_CACHE = {}


def _get_nc(with_biases):
    key = bool(with_biases)
    if key not in _CACHE:
        _CACHE[key] = build(key)
    return _CACHE[key]


def run(inputs, trace=False):
    q = np.asarray(inputs["q"], np.float32)
    k = np.asarray(inputs["k"], np.float32)
    v = np.asarray(inputs["v"], np.float32)
    B = q.shape[0]
    assert B == N_CORES and q.shape[1:] == (S, D)

    biases = np.stack(
        [
            np.asarray(inputs["Wq_b"], np.float32),
            np.asarray(inputs["Wk_b"], np.float32),
            np.asarray(inputs["Wv_b"], np.float32),
            np.asarray(inputs["out_b"], np.float32),
        ]
    )
    with_biases = bool(np.any(biases))

    shared = {
        "wqP": _pack_w(inputs["Wq_w"]),
        "wkP": _pack_w(inputs["Wk_w"]),
        "wvP": _pack_w(inputs["Wv_w"]),
        "woP": _pack_w(inputs["out_w"], "bfloat16"),
        "bandP": _pack_band(inputs["w_bias"], inputs["local_mask"]),
    }
    if with_biases:
        shared["biases"] = biases

    in_maps = []
    for b in range(B):
        m = dict(shared)
        kp = _pack_seq(k[b])
        vp = _pack_seq(v[b])
        m["kvP"] = np.ascontiguousarray(np.concatenate([kp, vp], axis=-1))
        m["qP"] = _pack_seq(q[b])
        in_maps.append(m)

    nc = _get_nc(with_biases)
    res = run_bass_kernel_spmd(
        nc, in_maps, core_ids=list(range(N_CORES)), trace=trace
    )
    out1 = np.stack([res.results[b]["out1"] for b in range(B)])
    num = np.stack([res.results[b]["num"] for b in range(B)])
    return (out1, num), res


def kernel(**inputs):
    (out1, num), _ = run(inputs, trace=False)
    return (out1, num)
